# revision 1
# baseline (speedup 1.0000x reference)
"""Multi-head attention (B=4, S=2048, D=1024, H=16) on 8 trn2 NeuronCores.

Sharding: data-parallel over batch (4) x tensor-parallel over heads (2 groups
of 8 heads). Core c handles batch b=c//2, head-group g=c%2. Each core:
  Q.T/K.T projections in [e, s] layout, V in [s, e] layout,
  scores computed transposed (S.T = K_h Q_h.T, [k, q]) so softmax needs no
  on-chip transposes; exp on ScalarE; row-sums via ones-vector matmuls;
  attn output accumulated in [e, q] layout which feeds the output projection
  directly. Per-core partial outputs are summed pairwise on the host.

Matmul dtype selectable: float32r (TF32: full PE rate at N=512, fp32-width
storage, host pre-rounds), bfloat16, or float32 (slow, exact).
"""

import numpy as np

import concourse.bass as bass
import concourse.bacc as bacc
import concourse.mybir as mybir
import concourse.tile as tile
from concourse.bass_utils import run_bass_kernel_spmd

# Problem constants (hardcoded per harness contract)
B, S, D = 4, 2048, 1024
NH, HDIM = 16, 64
NCORES = 8
EL = 512                 # per-core head columns (8 heads x 64)
NPAIR = 4                # head pairs per core
P = 128
QC = 512                 # q-chunk width (matmul N)
NQC = S // QC            # 4
NKT = S // P             # 16 k-tiles
NDT = D // P             # 8 d-tiles
NET = EL // P            # 4 e-tiles
KGRP = 2                 # k-tiles per exp group
F32 = mybir.dt.float32

_NC_CACHE = {}


def round_tf32(a):
    """Round fp32 array to tf32 (10-bit mantissa), round-to-nearest-even."""
    u = a.view(np.uint32)
    r = (u + np.uint32(0xFFF) + ((u >> np.uint32(13)) & np.uint32(1))) & np.uint32(0xFFFFE000)
    return r.view(np.float32)


def _host_cast(a, mdt):
    a = np.ascontiguousarray(np.asarray(a, dtype=np.float32))
    if mdt == mybir.dt.float32r:
        return round_tf32(a)
    if mdt == mybir.dt.bfloat16:
        import ml_dtypes
        return a.astype(ml_dtypes.bfloat16)
    return a


def build_nc(mdt=mybir.dt.float32r):
    nc = bacc.Bacc()
    xqT = nc.declare_dram_parameter("xqT", [D, S], mdt, isOutput=False)[:]
    xkT = nc.declare_dram_parameter("xkT", [D, S], mdt, isOutput=False)[:]
    xvT = nc.declare_dram_parameter("xvT", [D, S], mdt, isOutput=False)[:]
    wqT = nc.declare_dram_parameter("wqT", [D, EL], mdt, isOutput=False)[:]
    wkT = nc.declare_dram_parameter("wkT", [D, EL], mdt, isOutput=False)[:]
    wvT = nc.declare_dram_parameter("wvT", [D, EL], mdt, isOutput=False)[:]
    woT = nc.declare_dram_parameter("woT", [EL, D], mdt, isOutput=False)[:]
    bq = nc.declare_dram_parameter("bq", [EL], F32, isOutput=False)[:]
    bk = nc.declare_dram_parameter("bk", [EL], F32, isOutput=False)[:]
    bv = nc.declare_dram_parameter("bv", [EL], F32, isOutput=False)[:]
    outp = nc.declare_dram_parameter("outp", [S, D], F32, isOutput=True)[:]

    body = _body2 if mybir.dt.size(mdt) == 2 else _body
    with tile.TileContext(nc) as tc:
        body(nc, tc, mdt, xqT, xkT, xvT, wqT, wkT, wvT, woT, bq, bk, bv, outp)
    nc.finalize()
    return nc


def _body2(nc, tc, mdt, xqT, xkT, xvT, wqT, wkT, wvT, woT, bq, bk, bv, outp):
    """Single-phase layout for 2-byte matmul dtypes (everything fits SBUF).

    Order: K.T proj, V proj, then per q-chunk Q.T proj + attention + out
    proj, so ScalarE exp work starts as early as possible and the PE
    instruction stream stays dense (HAM stays warm).
    """
    from contextlib import ExitStack

    KG = 2  # k-tiles per score-psum tile; exp ops span [128, KG*QC]

    with ExitStack() as ctx:
        const = ctx.enter_context(tc.tile_pool(name="const", bufs=1))
        qkvp = ctx.enter_context(tc.tile_pool(name="qkvp", bufs=1))
        wpool = ctx.enter_context(tc.tile_pool(name="wpool", bufs=1))
        xpool = ctx.enter_context(tc.tile_pool(name="xpool", bufs=16))
        apool = ctx.enter_context(tc.tile_pool(name="apool", bufs=1))
        wo_pool = ctx.enter_context(tc.tile_pool(name="wo_pool", bufs=1))
        ptp = ctx.enter_context(tc.tile_pool(name="ptp", bufs=4))
        smallp = ctx.enter_context(tc.tile_pool(name="smallp", bufs=2))
        outsb = ctx.enter_context(tc.tile_pool(name="outsb", bufs=3))
        # PSUM: st 2 bufs x 2 banks + shared f32 tag 4 bufs x 1 bank = 8
        apsum = ctx.enter_context(tc.tile_pool(name="apsum", bufs=2, space="PSUM"))
        drp = ctx.enter_context(tc.tile_pool(name="drp", bufs=4, space="DRAM"))

        NHL = EL // HDIM
        VW = HDIM + 1
        ones_f = const.tile([P, NHL], F32)
        nc.vector.memset(ones_f, 1.0)
        bq_t, bk_t = [], []
        for et in range(NET):
            t1 = const.tile([P, 1], F32, tag=f"bq{et}", name=f"bq_t{et}")
            nc.sync.dma_start(out=t1, in_=bq[et * P:(et + 1) * P].rearrange("(p o) -> p o", o=1))
            bq_t.append(t1)
            t2 = const.tile([P, 1], F32, tag=f"bk{et}", name=f"bk_t{et}")
            nc.sync.dma_start(out=t2, in_=bk[et * P:(et + 1) * P].rearrange("(p o) -> p o", o=1))
            bk_t.append(t2)
        bvb = const.tile([P, EL], F32)
        nc.sync.dma_start(out=bvb, in_=bass.AP(tensor=bv.tensor, offset=bv.offset, ap=[[0, P], [1, EL]]))

        QT = [qkvp.tile([P, S], mdt, tag=f"qt{p}", name=f"QT{p}") for p in range(NPAIR)]
        KT = [qkvp.tile([P, S], mdt, tag=f"kt{p}", name=f"KT{p}") for p in range(NPAIR)]
        V = [qkvp.tile([P, NHL * VW], mdt, tag=f"v{i}", name=f"V{i}") for i in range(NKT)]
        ATT = [apool.tile([P, S], mdt, tag=f"att{p}", name=f"ATT{p}") for p in range(NPAIR)]

        wq_t, wk_t, wv_t = [], [], []
        for dt_i in range(NDT):
            for lst, src, nm in ((wk_t, wkT, "wk"), (wv_t, wvT, "wv"), (wq_t, wqT, "wq")):
                t = wpool.tile([P, EL], mdt, tag=f"{nm}{dt_i}", name=f"{nm}_t{dt_i}")
                nc.sync.dma_start(out=t, in_=src[dt_i * P:(dt_i + 1) * P, :])
                lst.append(t)
        WO = []
        for p in range(NPAIR):
            t = wo_pool.tile([P, D], mdt, tag=f"wo{p}", name=f"WO{p}")
            nc.sync.dma_start(out=t, in_=woT[p * P:(p + 1) * P, :])
            WO.append(t)

        def load_x(src, sc, nm):
            xt = []
            for dt_i in range(NDT):
                t = xpool.tile([P, QC], mdt, tag="x", name=f"{nm}_{sc}_{dt_i}")
                nc.sync.dma_start(out=t, in_=src[dt_i * P:(dt_i + 1) * P, sc * QC:(sc + 1) * QC])
                xt.append(t)
            return xt

        # ---- K.T projection ----
        for sc in range(NQC):
            ssl = slice(sc * QC, (sc + 1) * QC)
            xt = load_x(xkT, sc, "xk")
            for et in range(NET):
                ps = apsum.tile([P, QC], F32, tag="av", bufs=4, name=f"psk_{sc}_{et}")
                for dt_i in range(NDT):
                    nc.tensor.matmul(ps, wk_t[dt_i][:, et * P:(et + 1) * P], xt[dt_i],
                                     start=(dt_i == 0), stop=(dt_i == NDT - 1))
                nc.vector.tensor_scalar_add(out=KT[et][:, ssl], in0=ps, scalar1=bk_t[et])
        # ---- V projection (head-interleaved with ones column) ----
        for sc in range(NQC):
            xt = load_x(xvT, sc, "xv")
            for j in range(QC // P):
                ps = apsum.tile([P, EL], F32, tag="av", bufs=4, name=f"psv_{sc}_{j}")
                for dt_i in range(NDT):
                    nc.tensor.matmul(ps, xt[dt_i][:, j * P:(j + 1) * P], wv_t[dt_i],
                                     start=(dt_i == 0), stop=(dt_i == NDT - 1))
                vt = V[sc * (QC // P) + j]
                v3 = vt.rearrange("p (h c) -> p h c", c=VW)
                nc.vector.tensor_add(
                    out=v3[:, :, 0:HDIM],
                    in0=ps.rearrange("p (h c) -> p h c", c=HDIM),
                    in1=bvb.rearrange("p (h c) -> p h c", c=HDIM))
                nc.vector.tensor_copy(
                    out=v3[:, :, HDIM:VW],
                    in_=ones_f.rearrange("p (h o) -> p h o", o=1))

        EXPF = mybir.ActivationFunctionType.Exp
        scale = 1.0 / np.sqrt(HDIM)

        # ---- per q-chunk: Q.T projection, attention, out projection ----
        for qc in range(NQC):
            qsl = slice(qc * QC, (qc + 1) * QC)
            xt = load_x(xqT, qc, "xq")
            for et in range(NET):
                ps = apsum.tile([P, QC], F32, tag="av", bufs=4, name=f"psq_{qc}_{et}")
                for dt_i in range(NDT):
                    nc.tensor.matmul(ps, wq_t[dt_i][:, et * P:(et + 1) * P], xt[dt_i],
                                     start=(dt_i == 0), stop=(dt_i == NDT - 1))
                nc.vector.tensor_scalar_add(out=QT[et][:, qsl], in0=ps, scalar1=bq_t[et])

            for p in range(NPAIR):
                av = [apsum.tile([65, QC], F32, tag="av", bufs=4, name=f"av_{qc}_{p}_{h}") for h in range(2)]
                for r in range(NKT // KG):
                    kts = range(r * KG, (r + 1) * KG)
                    # ST row-pairs emitted adjacently (h0/h1 use
                    # disjoint PE row groups -> run concurrently)
                    st = [apsum.tile([P, KG * QC], F32, tag="st", name=f"st_{qc}_{p}_{r}_{h}") for h in range(2)]
                    for j, kt in enumerate(kts):
                        for h in range(2):
                            hsl = slice(h * 64, h * 64 + 64)
                            nc.tensor.matmul(
                                st[h][:, j * QC:(j + 1) * QC],
                                KT[p][hsl, kt * P:(kt + 1) * P],
                                QT[p][hsl, qsl],
                                start=True, stop=True,
                                tile_position=(h * 64, 0))
                    pt = []
                    for h in range(2):
                        ptt = ptp.tile([P, KG * QC], mdt, tag="pt", name=f"pt_{qc}_{p}_{r}_{h}")
                        nc.scalar.activation(out=ptt, in_=st[h], func=EXPF, scale=float(scale))
                        pt.append(ptt)
                    for j, kt in enumerate(kts):
                        for h in range(2):
                            hl = 2 * p + h
                            nc.tensor.matmul(
                                av[h],
                                V[kt][:, hl * VW:(hl + 1) * VW],
                                pt[h][:, j * QC:(j + 1) * QC],
                                start=(kt == 0), stop=(kt == NKT - 1))
                rc = smallp.tile([65, QC], F32, tag="rc", name=f"rc_{qc}_{p}")
                nc.vector.reciprocal(out=rc[64:65, :], in_=av[0][64:65, :])
                rc2 = smallp.tile([65, QC], F32, tag="rc2", name=f"rc2_{qc}_{p}")
                nc.vector.reciprocal(out=rc2[64:65, :], in_=av[1][64:65, :])
                dr = drp.tile([2, QC], F32, tag="dr", name=f"dr_{qc}_{p}")
                nc.sync.dma_start(out=dr[0:1, :], in_=rc[64:65, :])
                nc.sync.dma_start(out=dr[1:2, :], in_=rc2[64:65, :])
                rbc = smallp.tile([64, QC], F32, tag="rbc", name=f"rbc_{qc}_{p}")
                rbc2 = smallp.tile([64, QC], F32, tag="rbc2", name=f"rbc2_{qc}_{p}")
                d0, d1 = dr[0:1, :], dr[1:2, :]
                nc.sync.dma_start(
                    out=rbc,
                    in_=bass.AP(tensor=d0.tensor, offset=d0.offset, ap=[[0, 64], [1, QC]]))
                nc.sync.dma_start(
                    out=rbc2,
                    in_=bass.AP(tensor=d1.tensor, offset=d1.offset, ap=[[0, 64], [1, QC]]))
                nc.vector.tensor_mul(out=ATT[p][0:64, qsl], in0=av[0][0:64, :], in1=rbc)
                tmp1 = smallp.tile([64, QC], mdt, tag="tmp1", name=f"tmp1_{qc}_{p}")
                nc.vector.tensor_mul(out=tmp1, in0=av[1][0:64, :], in1=rbc2)
                nc.sync.dma_start(out=ATT[p][64:128, qsl], in_=tmp1)
            for st_i in range(QC // P):
                row = slice(qc * QC + st_i * P, qc * QC + (st_i + 1) * P)
                for dc in range(D // QC):
                    pso = apsum.tile([P, QC], F32, tag="av", bufs=4, name=f"pso_{qc}_{st_i}_{dc}")
                    for p in range(NPAIR):
                        nc.tensor.matmul(
                            pso, ATT[p][:, row], WO[p][:, dc * QC:(dc + 1) * QC],
                            start=(p == 0), stop=(p == NPAIR - 1))
                    ot = outsb.tile([P, QC], F32, tag="ot", name=f"ot_{qc}_{st_i}_{dc}")
                    nc.vector.tensor_copy(out=ot, in_=pso)
                    nc.sync.dma_start(out=outp[row, dc * QC:(dc + 1) * QC], in_=ot)


def _body(nc, tc, mdt, xqT, xkT, xvT, wqT, wkT, wvT, woT, bq, bk, bv, outp):
    from contextlib import ExitStack

    with ExitStack() as ctx:
        const = ctx.enter_context(tc.tile_pool(name="const", bufs=1))
        qkvp = ctx.enter_context(tc.tile_pool(name="qkvp", bufs=1))

        NHL = EL // HDIM          # 8 local heads
        VW = HDIM + 1             # 65: V columns per head incl. ones column
        ones_f = const.tile([P, NHL], F32)
        nc.vector.memset(ones_f, 1.0)

        # bias tiles: bq/bk as per-partition scalars per e-tile; bv broadcast
        bq_t, bk_t = [], []
        for et in range(NET):
            t1 = const.tile([P, 1], F32, tag=f"bq{et}", name=f"bq_t{et}")
            nc.sync.dma_start(out=t1, in_=bq[et * P:(et + 1) * P].rearrange("(p o) -> p o", o=1))
            bq_t.append(t1)
            t2 = const.tile([P, 1], F32, tag=f"bk{et}", name=f"bk_t{et}")
            nc.sync.dma_start(out=t2, in_=bk[et * P:(et + 1) * P].rearrange("(p o) -> p o", o=1))
            bk_t.append(t2)
        bvb = const.tile([P, EL], F32)
        nc.sync.dma_start(out=bvb, in_=bass.AP(tensor=bv.tensor, offset=bv.offset, ap=[[0, P], [1, EL]]))

        # persistent activations (matmul operands -> mdt storage)
        QT = [qkvp.tile([P, S], mdt, tag=f"qt{p}", name=f"QT{p}") for p in range(NPAIR)]
        KT = [qkvp.tile([P, S], mdt, tag=f"kt{p}", name=f"KT{p}") for p in range(NPAIR)]
        # V stored head-interleaved: per head 64 value cols + 1 ones col, so
        # the AV matmul (lhsT [k,65]) also produces the softmax denominator
        V = [qkvp.tile([P, NHL * VW], mdt, tag=f"v{i}", name=f"V{i}") for i in range(NKT)]

        # ---------------- projection phase ----------------
        with ExitStack() as pctx:
            wpool = pctx.enter_context(tc.tile_pool(name="wpool", bufs=1))
            xpool = pctx.enter_context(tc.tile_pool(name="xpool", bufs=16))
            ppsum = pctx.enter_context(tc.tile_pool(name="ppsum", bufs=4, space="PSUM"))

            wq_t, wk_t, wv_t = [], [], []
            for dt_i in range(NDT):
                for lst, src, nm in ((wq_t, wqT, "wq"), (wk_t, wkT, "wk"), (wv_t, wvT, "wv")):
                    t = wpool.tile([P, EL], mdt, tag=f"{nm}{dt_i}", name=f"{nm}_t{dt_i}")
                    nc.sync.dma_start(out=t, in_=src[dt_i * P:(dt_i + 1) * P, :])
                    lst.append(t)

            for sc in range(NQC):
                ssl = slice(sc * QC, (sc + 1) * QC)
                # Q.T and K.T: out [e-tile, s-chunk], lhsT = w tile, rhs = x.T chunk
                for (xsrc, wt, dst, bias) in ((xqT, wq_t, QT, bq_t), (xkT, wk_t, KT, bk_t)):
                    xt = []
                    for dt_i in range(NDT):
                        t = xpool.tile([P, QC], mdt, tag="x", name=f"x_{sc}_{dt_i}")
                        nc.sync.dma_start(out=t, in_=xsrc[dt_i * P:(dt_i + 1) * P, ssl])
                        xt.append(t)
                    for et in range(NET):
                        ps = ppsum.tile([P, QC], F32, tag="pp", name=f"ps_{sc}_{et}")
                        for dt_i in range(NDT):
                            nc.tensor.matmul(
                                ps, wt[dt_i][:, et * P:(et + 1) * P], xt[dt_i],
                                start=(dt_i == 0), stop=(dt_i == NDT - 1))
                        # pair tile p = et; copy + per-partition bias (rounds to mdt)
                        nc.vector.tensor_scalar_add(out=dst[et][:, ssl], in0=ps, scalar1=bias[et])
                # V: out [s-tile, e], lhsT = x.T chunk slice, rhs = w tile
                xt = []
                for dt_i in range(NDT):
                    t = xpool.tile([P, QC], mdt, tag="x", name=f"xv_{sc}_{dt_i}")
                    nc.sync.dma_start(out=t, in_=xvT[dt_i * P:(dt_i + 1) * P, ssl])
                    xt.append(t)
                for j in range(QC // P):
                    ps = ppsum.tile([P, EL], F32, tag="pp", name=f"psv_{sc}_{j}")
                    for dt_i in range(NDT):
                        nc.tensor.matmul(
                            ps, xt[dt_i][:, j * P:(j + 1) * P], wv_t[dt_i],
                            start=(dt_i == 0), stop=(dt_i == NDT - 1))
                    vt = V[sc * (QC // P) + j]
                    v3 = vt.rearrange("p (h c) -> p h c", c=VW)
                    nc.vector.tensor_add(
                        out=v3[:, :, 0:HDIM],
                        in0=ps.rearrange("p (h c) -> p h c", c=HDIM),
                        in1=bvb.rearrange("p (h c) -> p h c", c=HDIM))
                    nc.vector.tensor_copy(
                        out=v3[:, :, HDIM:VW],
                        in_=ones_f.rearrange("p (h o) -> p h o", o=1))

        # ---------------- attention + output projection ----------------
        apool = ctx.enter_context(tc.tile_pool(name="apool", bufs=1))
        wo_pool = ctx.enter_context(tc.tile_pool(name="wo_pool", bufs=1))
        ptp = ctx.enter_context(tc.tile_pool(name="ptp", bufs=4))
        smallp = ctx.enter_context(tc.tile_pool(name="smallp", bufs=2))
        outsb = ctx.enter_context(tc.tile_pool(name="outsb", bufs=3))
        apsum = ctx.enter_context(tc.tile_pool(name="apsum", bufs=2, space="PSUM"))
        # PSUM budget: st 2 bufs x 2 banks + av 4 bufs x 1 bank = 8 banks
        drp = ctx.enter_context(tc.tile_pool(name="drp", bufs=4, space="DRAM"))

        ATT = [apool.tile([P, S], mdt, tag=f"att{p}", name=f"ATT{p}") for p in range(NPAIR)]
        WO = []
        for p in range(NPAIR):
            t = wo_pool.tile([P, D], mdt, tag=f"wo{p}", name=f"WO{p}")
            nc.sync.dma_start(out=t, in_=woT[p * P:(p + 1) * P, :])
            WO.append(t)

        EXPF = mybir.ActivationFunctionType.Exp
        scale = 1.0 / np.sqrt(HDIM)

        for qc in range(NQC):
            qsl = slice(qc * QC, (qc + 1) * QC)
            for p in range(NPAIR):
                # per-head PSUM accumulators [65, QC]: rows 0-63 = attn@V,
                # row 64 = softmax denominator (from V's ones column)
                av = [apsum.tile([65, QC], F32, tag="av", bufs=4, name=f"av_{qc}_{p}_{h}") for h in range(2)]
                for r in range(NKT // KGRP):
                    kts = range(r * KGRP, (r + 1) * KGRP)
                    pt = []
                    for h in range(2):
                        hsl = slice(h * 64, h * 64 + 64)
                        st = apsum.tile([P, KGRP * QC], F32, tag="st", name=f"st_{qc}_{p}_{r}_{h}")
                        for j, kt in enumerate(kts):
                            nc.tensor.matmul(
                                st[:, j * QC:(j + 1) * QC],
                                KT[p][hsl, kt * P:(kt + 1) * P],
                                QT[p][hsl, qsl],
                                start=True, stop=True,
                                tile_position=(h * 64, 0))
                        ptt = ptp.tile([P, KGRP * QC], mdt, tag="pt", name=f"pt_{qc}_{p}_{r}_{h}")
                        nc.scalar.activation(out=ptt, in_=st, func=EXPF, scale=float(scale))
                        pt.append(ptt)
                    for j, kt in enumerate(kts):
                        for h in range(2):
                            hl = 2 * p + h
                            nc.tensor.matmul(
                                av[h],
                                V[kt][:, hl * VW:(hl + 1) * VW],
                                pt[h][:, j * QC:(j + 1) * QC],
                                start=(kt == 0), stop=(kt == NKT - 1))
                # normalize: recip of denominator row, broadcast to 64
                # partitions via a DRAM bounce (SBUF APs cannot have step-0
                # partition dims; DRAM APs can), then multiply
                rc = smallp.tile([65, QC], F32, tag="rc", name=f"rc_{qc}_{p}")
                nc.vector.reciprocal(out=rc[64:65, :], in_=av[0][64:65, :])
                rc2 = smallp.tile([65, QC], F32, tag="rc2", name=f"rc2_{qc}_{p}")
                nc.vector.reciprocal(out=rc2[64:65, :], in_=av[1][64:65, :])
                dr = drp.tile([2, QC], F32, tag="dr", name=f"dr_{qc}_{p}")
                nc.sync.dma_start(out=dr[0:1, :], in_=rc[64:65, :])
                nc.sync.dma_start(out=dr[1:2, :], in_=rc2[64:65, :])
                rbc = smallp.tile([64, QC], F32, tag="rbc", name=f"rbc_{qc}_{p}")
                rbc2 = smallp.tile([64, QC], F32, tag="rbc2", name=f"rbc2_{qc}_{p}")
                d0, d1 = dr[0:1, :], dr[1:2, :]
                nc.sync.dma_start(
                    out=rbc,
                    in_=bass.AP(tensor=d0.tensor, offset=d0.offset, ap=[[0, 64], [1, QC]]))
                nc.sync.dma_start(
                    out=rbc2,
                    in_=bass.AP(tensor=d1.tensor, offset=d1.offset, ap=[[0, 64], [1, QC]]))
                nc.vector.tensor_mul(out=ATT[p][0:64, qsl], in0=av[0][0:64, :], in1=rbc)
                # h1 rows belong at partitions 64-127 of ATT; DVE is
                # partition-locked, so normalize at 0-63 then DMA-shift
                tmp1 = smallp.tile([64, QC], mdt, tag="tmp1", name=f"tmp1_{qc}_{p}")
                nc.vector.tensor_mul(out=tmp1, in0=av[1][0:64, :], in1=rbc2)
                nc.sync.dma_start(out=ATT[p][64:128, qsl], in_=tmp1)
            # output projection for this q-chunk
            for st_i in range(QC // P):
                row = slice(qc * QC + st_i * P, qc * QC + (st_i + 1) * P)
                for dc in range(D // QC):
                    pso = apsum.tile([P, QC], F32, tag="av", bufs=4, name=f"pso_{qc}_{st_i}_{dc}")
                    for p in range(NPAIR):
                        nc.tensor.matmul(
                            pso, ATT[p][:, row], WO[p][:, dc * QC:(dc + 1) * QC],
                            start=(p == 0), stop=(p == NPAIR - 1))
                    ot = outsb.tile([P, QC], F32, tag="ot", name=f"ot_{qc}_{st_i}_{dc}")
                    nc.vector.tensor_copy(out=ot, in_=pso)
                    nc.sync.dma_start(out=outp[row, dc * QC:(dc + 1) * QC], in_=ot)


def make_in_maps(query, key, value, Wq, bq, Wk, bk, Wv, bv, Wo, bo,
                 mdt=mybir.dt.float32r):
    f32 = lambda a: np.ascontiguousarray(np.asarray(a, dtype=np.float32))
    in_maps = []
    for c in range(NCORES):
        b, g = c // 2, c % 2
        sl = slice(g * EL, (g + 1) * EL)
        in_maps.append({
            "xqT": _host_cast(np.asarray(query)[b].T, mdt),
            "xkT": _host_cast(np.asarray(key)[b].T, mdt),
            "xvT": _host_cast(np.asarray(value)[b].T, mdt),
            "wqT": _host_cast(np.asarray(Wq)[sl, :].T, mdt),
            "wkT": _host_cast(np.asarray(Wk)[sl, :].T, mdt),
            "wvT": _host_cast(np.asarray(Wv)[sl, :].T, mdt),
            "woT": _host_cast(np.asarray(Wo)[:, sl].T, mdt),
            "bq": f32(np.asarray(bq)[sl]),
            "bk": f32(np.asarray(bk)[sl]),
            "bv": f32(np.asarray(bv)[sl]),
        })
    return in_maps


def gather(results, bo):
    out = np.zeros((B, S, D), dtype=np.float32)
    for c in range(NCORES):
        out[c // 2] += results[c]["outp"]
    out += np.asarray(bo, dtype=np.float32)[None, None, :]
    return out


def run(inputs, trace=False, mdt=mybir.dt.float32r):
    key = str(mdt)
    if key not in _NC_CACHE:
        _NC_CACHE[key] = build_nc(mdt)
    nc = _NC_CACHE[key]
    in_maps = make_in_maps(**{k: inputs[k] for k in (
        "query", "key", "value", "Wq", "bq", "Wk", "bk", "Wv", "bv", "Wo", "bo")}, mdt=mdt)
    res = run_bass_kernel_spmd(nc, in_maps, list(range(NCORES)), trace=trace)
    return gather(res.results, inputs["bo"]), res


def kernel(**inputs):
    out, _ = run(inputs)
    return out



# revision 2
# speedup vs baseline: 1.1154x; 1.1154x over previous
"""Multi-head attention (B=4, S=2048, D=1024, H=16) on 8 trn2 NeuronCores.

Sharding: data-parallel over batch (4) x tensor-parallel over heads (2 groups
of 8 heads). Core c handles batch b=c//2, head-group g=c%2. Each core:
  Q.T/K.T projections in [e, s] layout, V in [s, e] layout,
  scores computed transposed (S.T = K_h Q_h.T, [k, q]) so softmax needs no
  on-chip transposes; exp on ScalarE; row-sums via ones-vector matmuls;
  attn output accumulated in [e, q] layout which feeds the output projection
  directly. Per-core partial outputs are summed pairwise on the host.

Matmul dtype selectable: float32r (TF32: full PE rate at N=512, fp32-width
storage, host pre-rounds), bfloat16, or float32 (slow, exact).
"""

import numpy as np

import concourse.bass as bass
import concourse.bacc as bacc
import concourse.mybir as mybir
import concourse.tile as tile
from concourse.bass_utils import run_bass_kernel_spmd

# Problem constants (hardcoded per harness contract)
B, S, D = 4, 2048, 1024
NH, HDIM = 16, 64
NCORES = 8
EL = 512                 # per-core head columns (8 heads x 64)
NPAIR = 4                # head pairs per core
P = 128
QC = 512                 # q-chunk width (matmul N)
NQC = S // QC            # 4
NKT = S // P             # 16 k-tiles
NDT = D // P             # 8 d-tiles
NET = EL // P            # 4 e-tiles
KGRP = 2                 # k-tiles per exp group
F32 = mybir.dt.float32

_NC_CACHE = {}


def round_tf32(a):
    """Round fp32 array to tf32 (10-bit mantissa), round-to-nearest-even."""
    u = a.view(np.uint32)
    r = (u + np.uint32(0xFFF) + ((u >> np.uint32(13)) & np.uint32(1))) & np.uint32(0xFFFFE000)
    return r.view(np.float32)


def _host_cast(a, mdt):
    a = np.ascontiguousarray(np.asarray(a, dtype=np.float32))
    if mdt == mybir.dt.float32r:
        return round_tf32(a)
    if mdt == mybir.dt.bfloat16:
        import ml_dtypes
        return a.astype(ml_dtypes.bfloat16)
    return a


def build_nc(mdt=mybir.dt.float32r):
    nc = bacc.Bacc()
    xqT = nc.declare_dram_parameter("xqT", [D, S], mdt, isOutput=False)[:]
    xkT = nc.declare_dram_parameter("xkT", [D, S], mdt, isOutput=False)[:]
    xvT = nc.declare_dram_parameter("xvT", [D, S], mdt, isOutput=False)[:]
    wqT = nc.declare_dram_parameter("wqT", [D, EL], mdt, isOutput=False)[:]
    wkT = nc.declare_dram_parameter("wkT", [D, EL], mdt, isOutput=False)[:]
    wvT = nc.declare_dram_parameter("wvT", [D, EL], mdt, isOutput=False)[:]
    woT = nc.declare_dram_parameter("woT", [EL, D], mdt, isOutput=False)[:]
    bq = nc.declare_dram_parameter("bq", [EL], F32, isOutput=False)[:]
    bk = nc.declare_dram_parameter("bk", [EL], F32, isOutput=False)[:]
    bv = nc.declare_dram_parameter("bv", [EL], F32, isOutput=False)[:]
    outp = nc.declare_dram_parameter("outp", [S, D], F32, isOutput=True)[:]

    from body3 import body3
    body = body3 if mybir.dt.size(mdt) == 2 else _body
    with tile.TileContext(nc) as tc:
        body(nc, tc, mdt, xqT, xkT, xvT, wqT, wkT, wvT, woT, bq, bk, bv, outp)
    nc.finalize()
    return nc


def _body2(nc, tc, mdt, xqT, xkT, xvT, wqT, wkT, wvT, woT, bq, bk, bv, outp):
    """Single-phase layout for 2-byte matmul dtypes (everything fits SBUF).

    Order: K.T proj, V proj, then per q-chunk Q.T proj + attention + out
    proj, so ScalarE exp work starts as early as possible and the PE
    instruction stream stays dense (HAM stays warm).
    """
    from contextlib import ExitStack

    KG = 2  # k-tiles per score-psum tile; exp ops span [128, KG*QC]

    with ExitStack() as ctx:
        const = ctx.enter_context(tc.tile_pool(name="const", bufs=1))
        qkvp = ctx.enter_context(tc.tile_pool(name="qkvp", bufs=1))
        wpool = ctx.enter_context(tc.tile_pool(name="wpool", bufs=1))
        xpool = ctx.enter_context(tc.tile_pool(name="xpool", bufs=16))
        apool = ctx.enter_context(tc.tile_pool(name="apool", bufs=1))
        wo_pool = ctx.enter_context(tc.tile_pool(name="wo_pool", bufs=1))
        ptp = ctx.enter_context(tc.tile_pool(name="ptp", bufs=4))
        smallp = ctx.enter_context(tc.tile_pool(name="smallp", bufs=2))
        outsb = ctx.enter_context(tc.tile_pool(name="outsb", bufs=3))
        # PSUM: st 2 bufs x 2 banks + shared f32 tag 4 bufs x 1 bank = 8
        apsum = ctx.enter_context(tc.tile_pool(name="apsum", bufs=2, space="PSUM"))
        drp = ctx.enter_context(tc.tile_pool(name="drp", bufs=4, space="DRAM"))

        NHL = EL // HDIM
        VW = HDIM + 1
        ones_f = const.tile([P, NHL], F32)
        nc.vector.memset(ones_f, 1.0)
        bq_t, bk_t = [], []
        for et in range(NET):
            t1 = const.tile([P, 1], F32, tag=f"bq{et}", name=f"bq_t{et}")
            nc.sync.dma_start(out=t1, in_=bq[et * P:(et + 1) * P].rearrange("(p o) -> p o", o=1))
            bq_t.append(t1)
            t2 = const.tile([P, 1], F32, tag=f"bk{et}", name=f"bk_t{et}")
            nc.sync.dma_start(out=t2, in_=bk[et * P:(et + 1) * P].rearrange("(p o) -> p o", o=1))
            bk_t.append(t2)
        bvb = const.tile([P, EL], F32)
        nc.sync.dma_start(out=bvb, in_=bass.AP(tensor=bv.tensor, offset=bv.offset, ap=[[0, P], [1, EL]]))

        QT = [qkvp.tile([P, S], mdt, tag=f"qt{p}", name=f"QT{p}") for p in range(NPAIR)]
        KT = [qkvp.tile([P, S], mdt, tag=f"kt{p}", name=f"KT{p}") for p in range(NPAIR)]
        V = [qkvp.tile([P, NHL * VW], mdt, tag=f"v{i}", name=f"V{i}") for i in range(NKT)]
        ATT = [apool.tile([P, S], mdt, tag=f"att{p}", name=f"ATT{p}") for p in range(NPAIR)]

        wq_t, wk_t, wv_t = [], [], []
        for dt_i in range(NDT):
            for lst, src, nm in ((wk_t, wkT, "wk"), (wv_t, wvT, "wv"), (wq_t, wqT, "wq")):
                t = wpool.tile([P, EL], mdt, tag=f"{nm}{dt_i}", name=f"{nm}_t{dt_i}")
                nc.sync.dma_start(out=t, in_=src[dt_i * P:(dt_i + 1) * P, :])
                lst.append(t)
        WO = []
        for p in range(NPAIR):
            t = wo_pool.tile([P, D], mdt, tag=f"wo{p}", name=f"WO{p}")
            nc.sync.dma_start(out=t, in_=woT[p * P:(p + 1) * P, :])
            WO.append(t)

        def load_x(src, sc, nm):
            xt = []
            for dt_i in range(NDT):
                t = xpool.tile([P, QC], mdt, tag="x", name=f"{nm}_{sc}_{dt_i}")
                nc.sync.dma_start(out=t, in_=src[dt_i * P:(dt_i + 1) * P, sc * QC:(sc + 1) * QC])
                xt.append(t)
            return xt

        # ---- K.T projection ----
        for sc in range(NQC):
            ssl = slice(sc * QC, (sc + 1) * QC)
            xt = load_x(xkT, sc, "xk")
            for et in range(NET):
                ps = apsum.tile([P, QC], F32, tag="av", bufs=4, name=f"psk_{sc}_{et}")
                for dt_i in range(NDT):
                    nc.tensor.matmul(ps, wk_t[dt_i][:, et * P:(et + 1) * P], xt[dt_i],
                                     start=(dt_i == 0), stop=(dt_i == NDT - 1))
                nc.vector.tensor_scalar_add(out=KT[et][:, ssl], in0=ps, scalar1=bk_t[et])
        # ---- V projection (head-interleaved with ones column) ----
        for sc in range(NQC):
            xt = load_x(xvT, sc, "xv")
            for j in range(QC // P):
                ps = apsum.tile([P, EL], F32, tag="av", bufs=4, name=f"psv_{sc}_{j}")
                for dt_i in range(NDT):
                    nc.tensor.matmul(ps, xt[dt_i][:, j * P:(j + 1) * P], wv_t[dt_i],
                                     start=(dt_i == 0), stop=(dt_i == NDT - 1))
                vt = V[sc * (QC // P) + j]
                v3 = vt.rearrange("p (h c) -> p h c", c=VW)
                nc.vector.tensor_add(
                    out=v3[:, :, 0:HDIM],
                    in0=ps.rearrange("p (h c) -> p h c", c=HDIM),
                    in1=bvb.rearrange("p (h c) -> p h c", c=HDIM))
                nc.vector.tensor_copy(
                    out=v3[:, :, HDIM:VW],
                    in_=ones_f.rearrange("p (h o) -> p h o", o=1))

        EXPF = mybir.ActivationFunctionType.Exp
        scale = 1.0 / np.sqrt(HDIM)

        # ---- per q-chunk: Q.T projection, attention, out projection ----
        for qc in range(NQC):
            qsl = slice(qc * QC, (qc + 1) * QC)
            xt = load_x(xqT, qc, "xq")
            for et in range(NET):
                ps = apsum.tile([P, QC], F32, tag="av", bufs=4, name=f"psq_{qc}_{et}")
                for dt_i in range(NDT):
                    nc.tensor.matmul(ps, wq_t[dt_i][:, et * P:(et + 1) * P], xt[dt_i],
                                     start=(dt_i == 0), stop=(dt_i == NDT - 1))
                nc.vector.tensor_scalar_add(out=QT[et][:, qsl], in0=ps, scalar1=bq_t[et])

            for p in range(NPAIR):
                av = [apsum.tile([65, QC], F32, tag="av", bufs=4, name=f"av_{qc}_{p}_{h}") for h in range(2)]
                for r in range(NKT // KG):
                    kts = range(r * KG, (r + 1) * KG)
                    # ST row-pairs emitted adjacently (h0/h1 use
                    # disjoint PE row groups -> run concurrently)
                    st = [apsum.tile([P, KG * QC], F32, tag="st", name=f"st_{qc}_{p}_{r}_{h}") for h in range(2)]
                    for j, kt in enumerate(kts):
                        for h in range(2):
                            hsl = slice(h * 64, h * 64 + 64)
                            nc.tensor.matmul(
                                st[h][:, j * QC:(j + 1) * QC],
                                KT[p][hsl, kt * P:(kt + 1) * P],
                                QT[p][hsl, qsl],
                                start=True, stop=True,
                                tile_position=(h * 64, 0))
                    pt = []
                    for h in range(2):
                        ptt = ptp.tile([P, KG * QC], mdt, tag="pt", name=f"pt_{qc}_{p}_{r}_{h}")
                        nc.scalar.activation(out=ptt, in_=st[h], func=EXPF, scale=float(scale))
                        pt.append(ptt)
                    for j, kt in enumerate(kts):
                        for h in range(2):
                            hl = 2 * p + h
                            nc.tensor.matmul(
                                av[h],
                                V[kt][:, hl * VW:(hl + 1) * VW],
                                pt[h][:, j * QC:(j + 1) * QC],
                                start=(kt == 0), stop=(kt == NKT - 1))
                rc = smallp.tile([65, QC], F32, tag="rc", name=f"rc_{qc}_{p}")
                nc.vector.reciprocal(out=rc[64:65, :], in_=av[0][64:65, :])
                rc2 = smallp.tile([65, QC], F32, tag="rc2", name=f"rc2_{qc}_{p}")
                nc.vector.reciprocal(out=rc2[64:65, :], in_=av[1][64:65, :])
                dr = drp.tile([2, QC], F32, tag="dr", name=f"dr_{qc}_{p}")
                nc.sync.dma_start(out=dr[0:1, :], in_=rc[64:65, :])
                nc.sync.dma_start(out=dr[1:2, :], in_=rc2[64:65, :])
                rbc = smallp.tile([64, QC], F32, tag="rbc", name=f"rbc_{qc}_{p}")
                rbc2 = smallp.tile([64, QC], F32, tag="rbc2", name=f"rbc2_{qc}_{p}")
                d0, d1 = dr[0:1, :], dr[1:2, :]
                nc.sync.dma_start(
                    out=rbc,
                    in_=bass.AP(tensor=d0.tensor, offset=d0.offset, ap=[[0, 64], [1, QC]]))
                nc.sync.dma_start(
                    out=rbc2,
                    in_=bass.AP(tensor=d1.tensor, offset=d1.offset, ap=[[0, 64], [1, QC]]))
                nc.vector.tensor_mul(out=ATT[p][0:64, qsl], in0=av[0][0:64, :], in1=rbc)
                tmp1 = smallp.tile([64, QC], mdt, tag="tmp1", name=f"tmp1_{qc}_{p}")
                nc.vector.tensor_mul(out=tmp1, in0=av[1][0:64, :], in1=rbc2)
                nc.sync.dma_start(out=ATT[p][64:128, qsl], in_=tmp1)
            for st_i in range(QC // P):
                row = slice(qc * QC + st_i * P, qc * QC + (st_i + 1) * P)
                for dc in range(D // QC):
                    pso = apsum.tile([P, QC], F32, tag="av", bufs=4, name=f"pso_{qc}_{st_i}_{dc}")
                    for p in range(NPAIR):
                        nc.tensor.matmul(
                            pso, ATT[p][:, row], WO[p][:, dc * QC:(dc + 1) * QC],
                            start=(p == 0), stop=(p == NPAIR - 1))
                    ot = outsb.tile([P, QC], F32, tag="ot", name=f"ot_{qc}_{st_i}_{dc}")
                    nc.vector.tensor_copy(out=ot, in_=pso)
                    nc.sync.dma_start(out=outp[row, dc * QC:(dc + 1) * QC], in_=ot)


def _body(nc, tc, mdt, xqT, xkT, xvT, wqT, wkT, wvT, woT, bq, bk, bv, outp):
    from contextlib import ExitStack

    with ExitStack() as ctx:
        const = ctx.enter_context(tc.tile_pool(name="const", bufs=1))
        qkvp = ctx.enter_context(tc.tile_pool(name="qkvp", bufs=1))

        NHL = EL // HDIM          # 8 local heads
        VW = HDIM + 1             # 65: V columns per head incl. ones column
        ones_f = const.tile([P, NHL], F32)
        nc.vector.memset(ones_f, 1.0)

        # bias tiles: bq/bk as per-partition scalars per e-tile; bv broadcast
        bq_t, bk_t = [], []
        for et in range(NET):
            t1 = const.tile([P, 1], F32, tag=f"bq{et}", name=f"bq_t{et}")
            nc.sync.dma_start(out=t1, in_=bq[et * P:(et + 1) * P].rearrange("(p o) -> p o", o=1))
            bq_t.append(t1)
            t2 = const.tile([P, 1], F32, tag=f"bk{et}", name=f"bk_t{et}")
            nc.sync.dma_start(out=t2, in_=bk[et * P:(et + 1) * P].rearrange("(p o) -> p o", o=1))
            bk_t.append(t2)
        bvb = const.tile([P, EL], F32)
        nc.sync.dma_start(out=bvb, in_=bass.AP(tensor=bv.tensor, offset=bv.offset, ap=[[0, P], [1, EL]]))

        # persistent activations (matmul operands -> mdt storage)
        QT = [qkvp.tile([P, S], mdt, tag=f"qt{p}", name=f"QT{p}") for p in range(NPAIR)]
        KT = [qkvp.tile([P, S], mdt, tag=f"kt{p}", name=f"KT{p}") for p in range(NPAIR)]
        # V stored head-interleaved: per head 64 value cols + 1 ones col, so
        # the AV matmul (lhsT [k,65]) also produces the softmax denominator
        V = [qkvp.tile([P, NHL * VW], mdt, tag=f"v{i}", name=f"V{i}") for i in range(NKT)]

        # ---------------- projection phase ----------------
        with ExitStack() as pctx:
            wpool = pctx.enter_context(tc.tile_pool(name="wpool", bufs=1))
            xpool = pctx.enter_context(tc.tile_pool(name="xpool", bufs=16))
            ppsum = pctx.enter_context(tc.tile_pool(name="ppsum", bufs=4, space="PSUM"))

            wq_t, wk_t, wv_t = [], [], []
            for dt_i in range(NDT):
                for lst, src, nm in ((wq_t, wqT, "wq"), (wk_t, wkT, "wk"), (wv_t, wvT, "wv")):
                    t = wpool.tile([P, EL], mdt, tag=f"{nm}{dt_i}", name=f"{nm}_t{dt_i}")
                    nc.sync.dma_start(out=t, in_=src[dt_i * P:(dt_i + 1) * P, :])
                    lst.append(t)

            for sc in range(NQC):
                ssl = slice(sc * QC, (sc + 1) * QC)
                # Q.T and K.T: out [e-tile, s-chunk], lhsT = w tile, rhs = x.T chunk
                for (xsrc, wt, dst, bias) in ((xqT, wq_t, QT, bq_t), (xkT, wk_t, KT, bk_t)):
                    xt = []
                    for dt_i in range(NDT):
                        t = xpool.tile([P, QC], mdt, tag="x", name=f"x_{sc}_{dt_i}")
                        nc.sync.dma_start(out=t, in_=xsrc[dt_i * P:(dt_i + 1) * P, ssl])
                        xt.append(t)
                    for et in range(NET):
                        ps = ppsum.tile([P, QC], F32, tag="pp", name=f"ps_{sc}_{et}")
                        for dt_i in range(NDT):
                            nc.tensor.matmul(
                                ps, wt[dt_i][:, et * P:(et + 1) * P], xt[dt_i],
                                start=(dt_i == 0), stop=(dt_i == NDT - 1))
                        # pair tile p = et; copy + per-partition bias (rounds to mdt)
                        nc.vector.tensor_scalar_add(out=dst[et][:, ssl], in0=ps, scalar1=bias[et])
                # V: out [s-tile, e], lhsT = x.T chunk slice, rhs = w tile
                xt = []
                for dt_i in range(NDT):
                    t = xpool.tile([P, QC], mdt, tag="x", name=f"xv_{sc}_{dt_i}")
                    nc.sync.dma_start(out=t, in_=xvT[dt_i * P:(dt_i + 1) * P, ssl])
                    xt.append(t)
                for j in range(QC // P):
                    ps = ppsum.tile([P, EL], F32, tag="pp", name=f"psv_{sc}_{j}")
                    for dt_i in range(NDT):
                        nc.tensor.matmul(
                            ps, xt[dt_i][:, j * P:(j + 1) * P], wv_t[dt_i],
                            start=(dt_i == 0), stop=(dt_i == NDT - 1))
                    vt = V[sc * (QC // P) + j]
                    v3 = vt.rearrange("p (h c) -> p h c", c=VW)
                    nc.vector.tensor_add(
                        out=v3[:, :, 0:HDIM],
                        in0=ps.rearrange("p (h c) -> p h c", c=HDIM),
                        in1=bvb.rearrange("p (h c) -> p h c", c=HDIM))
                    nc.vector.tensor_copy(
                        out=v3[:, :, HDIM:VW],
                        in_=ones_f.rearrange("p (h o) -> p h o", o=1))

        # ---------------- attention + output projection ----------------
        apool = ctx.enter_context(tc.tile_pool(name="apool", bufs=1))
        wo_pool = ctx.enter_context(tc.tile_pool(name="wo_pool", bufs=1))
        ptp = ctx.enter_context(tc.tile_pool(name="ptp", bufs=4))
        smallp = ctx.enter_context(tc.tile_pool(name="smallp", bufs=2))
        outsb = ctx.enter_context(tc.tile_pool(name="outsb", bufs=3))
        apsum = ctx.enter_context(tc.tile_pool(name="apsum", bufs=2, space="PSUM"))
        # PSUM budget: st 2 bufs x 2 banks + av 4 bufs x 1 bank = 8 banks
        drp = ctx.enter_context(tc.tile_pool(name="drp", bufs=4, space="DRAM"))

        ATT = [apool.tile([P, S], mdt, tag=f"att{p}", name=f"ATT{p}") for p in range(NPAIR)]
        WO = []
        for p in range(NPAIR):
            t = wo_pool.tile([P, D], mdt, tag=f"wo{p}", name=f"WO{p}")
            nc.sync.dma_start(out=t, in_=woT[p * P:(p + 1) * P, :])
            WO.append(t)

        EXPF = mybir.ActivationFunctionType.Exp
        scale = 1.0 / np.sqrt(HDIM)

        for qc in range(NQC):
            qsl = slice(qc * QC, (qc + 1) * QC)
            for p in range(NPAIR):
                # per-head PSUM accumulators [65, QC]: rows 0-63 = attn@V,
                # row 64 = softmax denominator (from V's ones column)
                av = [apsum.tile([65, QC], F32, tag="av", bufs=4, name=f"av_{qc}_{p}_{h}") for h in range(2)]
                for r in range(NKT // KGRP):
                    kts = range(r * KGRP, (r + 1) * KGRP)
                    pt = []
                    for h in range(2):
                        hsl = slice(h * 64, h * 64 + 64)
                        st = apsum.tile([P, KGRP * QC], F32, tag="st", name=f"st_{qc}_{p}_{r}_{h}")
                        for j, kt in enumerate(kts):
                            nc.tensor.matmul(
                                st[:, j * QC:(j + 1) * QC],
                                KT[p][hsl, kt * P:(kt + 1) * P],
                                QT[p][hsl, qsl],
                                start=True, stop=True,
                                tile_position=(h * 64, 0))
                        ptt = ptp.tile([P, KGRP * QC], mdt, tag="pt", name=f"pt_{qc}_{p}_{r}_{h}")
                        nc.scalar.activation(out=ptt, in_=st, func=EXPF, scale=float(scale))
                        pt.append(ptt)
                    for j, kt in enumerate(kts):
                        for h in range(2):
                            hl = 2 * p + h
                            nc.tensor.matmul(
                                av[h],
                                V[kt][:, hl * VW:(hl + 1) * VW],
                                pt[h][:, j * QC:(j + 1) * QC],
                                start=(kt == 0), stop=(kt == NKT - 1))
                # normalize: recip of denominator row, broadcast to 64
                # partitions via a DRAM bounce (SBUF APs cannot have step-0
                # partition dims; DRAM APs can), then multiply
                rc = smallp.tile([65, QC], F32, tag="rc", name=f"rc_{qc}_{p}")
                nc.vector.reciprocal(out=rc[64:65, :], in_=av[0][64:65, :])
                rc2 = smallp.tile([65, QC], F32, tag="rc2", name=f"rc2_{qc}_{p}")
                nc.vector.reciprocal(out=rc2[64:65, :], in_=av[1][64:65, :])
                dr = drp.tile([2, QC], F32, tag="dr", name=f"dr_{qc}_{p}")
                nc.sync.dma_start(out=dr[0:1, :], in_=rc[64:65, :])
                nc.sync.dma_start(out=dr[1:2, :], in_=rc2[64:65, :])
                rbc = smallp.tile([64, QC], F32, tag="rbc", name=f"rbc_{qc}_{p}")
                rbc2 = smallp.tile([64, QC], F32, tag="rbc2", name=f"rbc2_{qc}_{p}")
                d0, d1 = dr[0:1, :], dr[1:2, :]
                nc.sync.dma_start(
                    out=rbc,
                    in_=bass.AP(tensor=d0.tensor, offset=d0.offset, ap=[[0, 64], [1, QC]]))
                nc.sync.dma_start(
                    out=rbc2,
                    in_=bass.AP(tensor=d1.tensor, offset=d1.offset, ap=[[0, 64], [1, QC]]))
                nc.vector.tensor_mul(out=ATT[p][0:64, qsl], in0=av[0][0:64, :], in1=rbc)
                # h1 rows belong at partitions 64-127 of ATT; DVE is
                # partition-locked, so normalize at 0-63 then DMA-shift
                tmp1 = smallp.tile([64, QC], mdt, tag="tmp1", name=f"tmp1_{qc}_{p}")
                nc.vector.tensor_mul(out=tmp1, in0=av[1][0:64, :], in1=rbc2)
                nc.sync.dma_start(out=ATT[p][64:128, qsl], in_=tmp1)
            # output projection for this q-chunk
            for st_i in range(QC // P):
                row = slice(qc * QC + st_i * P, qc * QC + (st_i + 1) * P)
                for dc in range(D // QC):
                    pso = apsum.tile([P, QC], F32, tag="av", bufs=4, name=f"pso_{qc}_{st_i}_{dc}")
                    for p in range(NPAIR):
                        nc.tensor.matmul(
                            pso, ATT[p][:, row], WO[p][:, dc * QC:(dc + 1) * QC],
                            start=(p == 0), stop=(p == NPAIR - 1))
                    ot = outsb.tile([P, QC], F32, tag="ot", name=f"ot_{qc}_{st_i}_{dc}")
                    nc.vector.tensor_copy(out=ot, in_=pso)
                    nc.sync.dma_start(out=outp[row, dc * QC:(dc + 1) * QC], in_=ot)


def make_in_maps(query, key, value, Wq, bq, Wk, bk, Wv, bv, Wo, bo,
                 mdt=mybir.dt.float32r):
    f32 = lambda a: np.ascontiguousarray(np.asarray(a, dtype=np.float32))
    in_maps = []
    for c in range(NCORES):
        b, g = c // 2, c % 2
        sl = slice(g * EL, (g + 1) * EL)
        in_maps.append({
            "xqT": _host_cast(np.asarray(query)[b].T, mdt),
            "xkT": _host_cast(np.asarray(key)[b].T, mdt),
            "xvT": _host_cast(np.asarray(value)[b].T, mdt),
            "wqT": _host_cast(np.asarray(Wq)[sl, :].T, mdt),
            "wkT": _host_cast(np.asarray(Wk)[sl, :].T, mdt),
            "wvT": _host_cast(np.asarray(Wv)[sl, :].T, mdt),
            "woT": _host_cast(np.asarray(Wo)[:, sl].T, mdt),
            "bq": f32(np.asarray(bq)[sl]),
            "bk": f32(np.asarray(bk)[sl]),
            "bv": f32(np.asarray(bv)[sl]),
        })
    return in_maps


def gather(results, bo):
    out = np.zeros((B, S, D), dtype=np.float32)
    for c in range(NCORES):
        out[c // 2] += results[c]["outp"]
    out += np.asarray(bo, dtype=np.float32)[None, None, :]
    return out


def run(inputs, trace=False, mdt=mybir.dt.float32r):
    key = str(mdt)
    if key not in _NC_CACHE:
        _NC_CACHE[key] = build_nc(mdt)
    nc = _NC_CACHE[key]
    in_maps = make_in_maps(**{k: inputs[k] for k in (
        "query", "key", "value", "Wq", "bq", "Wk", "bk", "Wv", "bv", "Wo", "bo")}, mdt=mdt)
    res = run_bass_kernel_spmd(nc, in_maps, list(range(NCORES)), trace=trace)
    return gather(res.results, inputs["bo"]), res


def kernel(**inputs):
    out, _ = run(inputs)
    return out

